# revision 33
# baseline (speedup 1.0000x reference)
"""Trainium2 Bass kernel for nn_Attention (B=8, Sq=Skv=2048, d=512).

Sharding: data-parallel over batch -- core b handles batch b (8 cores).

Per-core pipeline v4 (PE runs matmuls ONLY; every transpose is a DMA
XBAR op; QK entirely in fp8 DoubleRow):

  The tensor engine's instruction stream is pure projection / QK / PV
  matmuls.  All 128x128 transposes (q/k feature-major conversion in
  stage 1, P^T in stage 2) run on the DMA engines' hardware XBAR
  (InstDmaTransposeAnt, bf16, 14ns per 16x128 tile), issued from the
  SP queue with a 2-3 tile emission lag so their data-ready semaphore
  waits are always stale-satisfied and never head-block the queue that
  also streams the x^T input tiles.

  stage 1 (per 128-row tile of ques/keys/vals):
    DMA in the host-pre-transposed bf16 x^T tile -> 4 projection
    matmuls (PSUM ring) -> for q/k: layernorm stats on DVE, rstd on ACT
    via exp(-0.5*ln(var+eps) + ln(gain*scale)) (uniform LN gain and the
    1/sqrt(dk) softmax scale fold into the Exp bias; only ln/exp/copy/
    identity tables are used -> one act table load total) -> one fused
    (pr*rstd + c1) PSUM eviction to bf16 y -> XBAR-transpose y into a
    staging yT -> one lagged DVE/ACT copy converts yT to the persistent
    fp8 qT8/kT8 [d_part, d_chunk, seq].  v rows evicted to bf16 SBUF.
    LN biases are handled exactly: b_k shifts every logit row by a
    per-query constant (softmax-invariant, dropped); b_q != 0 falls back.

  stage 2 (per 128-row query tile t; causal: kv <= 128(t+1)):
    S chunks = qT8.T @ kT8 (fp8 DoubleRow, 2 matmuls per 512-wide
    chunk -> fp32 PSUM, ring 2) -> exp on ACT with fused row-sum
    accumulation (no max subtraction: |S| <= sqrt(dk)*g^2 since q/k are
    layernormed); the diagonal block's strictly-upper triangle is
    zeroed multiplicatively on DVE -> XBAR-transpose P 128-blocks into
    pt tiles (DMA, overlapped with the next tile's S matmuls) -> PV
    matmuls one query-tile behind the S matmuls -> fused (o/rowsum +
    residual) on DVE -> output layernorm -> DMA out (fp32).
"""

import math
import numpy as np

B = 8
S = 2048
D = 512
P = 128
KC = D // P       # 4 feature chunks
NT = S // P       # 16 seq tiles
EPS = 1e-5
NEG = np.float32(-1e30)

_CACHE = {}


def _bf16np():
    from concourse import mybir
    return mybir.dt.np(mybir.dt.bfloat16)


def _round_f32r(a):
    """Round fp32 to the PE's f32r grid (kept for tooling compatibility)."""
    b = np.ascontiguousarray(a, np.float32).view(np.uint32).astype(np.int64)
    low = b & 0xFFF
    base = b & ~np.int64(0xFFF)
    up = base + 0x1000
    r = np.where(low > 0x800, up,
                 np.where(low < 0x800, base,
                          np.where((base >> 12) & 1, up, base)))
    return r.astype(np.uint32).view(np.float32).reshape(a.shape)


def _build(has_km, loop_n=0, has_gobo=False):
    from contextlib import ExitStack

    import concourse.tile as tile
    from concourse import bacc, mybir

    f32 = mybir.dt.float32
    bf16 = mybir.dt.bfloat16
    fp8 = mybir.dt.float8e4
    DR = mybir.MatmulPerfMode.DoubleRow
    Alu = mybir.AluOpType
    Act = mybir.ActivationFunctionType

    class OneActSetBacc(bacc.Bacc):
        """Force every activation onto the ln+exp+copy+identity table set.

        The default chooser maps each function to the first act-func-set
        containing it (Exp -> set 0, Ln -> set 5), which makes alternating
        ln/exp insert a ~1.3us table load per pair.  This kernel only uses
        functions that all live in 'natural_log_exp_and_others', so empty
        out the earlier sets; the fixpoint pass then emits one load total.
        """

        def insert_act_table_loads(self):
            import bass_rust as _bass_rust
            from concourse.hw_specs import get_activation_tables

            has_activation = any(
                isinstance(i, mybir.InstActivation)
                for b in self.main_func.blocks
                for i in b.instructions
            )
            if not has_activation:
                return
            tables = list(get_activation_tables(self.m.arch).items())
            target = next(i for i, (n, _) in enumerate(tables)
                          if n == "natural_log_exp_and_others")
            tables = [(n, (s if i >= target else set()))
                      for i, (n, s) in enumerate(tables)]
            _bass_rust.insert_act_table_loads(self, tables)

    nc = OneActSetBacc("TRN2", target_bir_lowering=False, debug=False,
                       num_devices=B)

    # Inputs are host-packed partition-major so each full tensor loads in
    # ONE DMA of 128 contiguous 16KB descriptors (each HWDGE op pays a
    # ~625ns fixed descriptor-generation cost, so DMA count is minimized):
    #   xT  [P, NT*D]: col (i*D + c*P + s') holds x[i*P + s', c*P + p]
    #   xq  [P, NT*D]: col (i*D + d)        holds ques[i*P + p, d]
    xq_d = nc.dram_tensor("xq", [P, NT * D], f32, kind="ExternalInput").ap()
    xqT_d = nc.dram_tensor("xqT", [P, NT * D], bf16, kind="ExternalInput").ap()
    xkT_d = nc.dram_tensor("xkT", [P, NT * D], bf16, kind="ExternalInput").ap()
    xvT_d = nc.dram_tensor("xvT", [P, NT * D], bf16, kind="ExternalInput").ap()
    wq_d = nc.dram_tensor("wq", [D, D], bf16, kind="ExternalInput").ap()
    wk_d = nc.dram_tensor("wk", [D, D], bf16, kind="ExternalInput").ap()
    wv_d = nc.dram_tensor("wv", [D, D], bf16, kind="ExternalInput").ap()
    # lnb: [:,0]=ln(g_q/sqrt(dk)), [:,1]=ln(g_k), [:,2]=ln(g_o or 1)
    lnb_d = nc.dram_tensor("lnb", [P, 3], f32, kind="ExternalInput").ap()
    gobo_d = nc.dram_tensor("gobo", [P, 2 * D], f32, kind="ExternalInput").ap()
    km_d = nc.dram_tensor("km", [P, S], f32, kind="ExternalInput").ap()
    out_d = nc.dram_tensor("out", [S, D], f32, kind="ExternalOutput").ap()

    with tile.TileContext(nc) as tc, ExitStack() as ctx:
        cpool = ctx.enter_context(tc.tile_pool(name="consts", bufs=1))
        y_pool = ctx.enter_context(tc.tile_pool(name="ypool", bufs=6))
        yt_pool = ctx.enter_context(tc.tile_pool(name="ytpool", bufs=6))
        small = ctx.enter_context(tc.tile_pool(name="small", bufs=8))
        p_pool = ctx.enter_context(tc.tile_pool(name="ppool", bufs=3))
        pt_pool = ctx.enter_context(tc.tile_pool(name="ptpool", bufs=3))
        z_pool = ctx.enter_context(tc.tile_pool(name="zpool", bufs=3))
        big = ctx.enter_context(tc.tile_pool(name="big", bufs=1))

        lnb = cpool.tile([P, 3], f32)
        nc.sync.dma_start(lnb[:], lnb_d)
        eps_sb = cpool.tile([P, 1], f32)
        nc.vector.memset(eps_sb[:], EPS)

        # whole-tensor input buffers (loaded in ONE DMA each at the top of
        # the loop body; across loop iterations the loads overlap the
        # previous iteration's stage 2 -- the WAR reuse semaphores release
        # at stage-1 end for the xT streams and at the final residual read
        # for xq)
        xin = {}
        for nm in ("xqT", "xkT", "xvT"):
            xin_t = big.tile([P, NT * D], bf16, tag=nm, name=nm + "_sb")
            xin[nm] = xin_t
        xq_sb = big.tile([P, NT * D], f32, tag="xq")
        xin_dram = {"xqT": xqT_d, "xkT": xkT_d, "xvT": xvT_d}

        # weights (bf16, host-converted): [d_in_part, d_in_chunk, d_out]
        w_r = {}
        for name, dram in (("wq", wq_d), ("wk", wk_d), ("wv", wv_d)):
            wr = cpool.tile([P, KC, D], bf16, tag=name + "r")
            nc.sync.dma_start(wr[:], dram.rearrange("(c p) n -> p c n", p=P))
            w_r[name] = wr

        if has_gobo:
            gobo = cpool.tile([P, 2 * D], f32)
            nc.sync.dma_start(gobo[:], gobo_d)
            go_sb = gobo[:, 0:D]
            bo_sb = gobo[:, D:2 * D]
        if has_km:
            km = cpool.tile([P, S], f32)
            nc.sync.dma_start(km[:], km_d)

        # persistent per-batch tensors (QK reads fp8 only); q and k share
        # one tensor: chunks 0..3 = q, 4..7 = k  [d_part, dchunk, seq]
        q8k8 = big.tile([P, 2 * KC, S], fp8, tag="q8k8")
        v_sb = big.tile([P, NT, D], bf16, tag="v")      # [kv_part, kvtile, dv]

        expb = cpool.tile([P, 1], f32)
        nc.vector.memset(expb[:], -1.5)
        ltri_f = cpool.tile([P, P], f32)
        nc.gpsimd.memset(ltri_f[:], 1.0)
        nc.gpsimd.affine_select(ltri_f[:], ltri_f[:],
                                pattern=[[-1, P]], base=0, channel_multiplier=1,
                                compare_op=mybir.AluOpType.is_ge, fill=0.0)
        ltri01 = cpool.tile([P, P], bf16)
        nc.vector.tensor_copy(ltri01[:], ltri_f[:])

        # PSUM: proj ring 3 + S ring 3 + O 2 = 8
        proj_ps = ctx.enter_context(tc.tile_pool(name="proj_ps", bufs=3,
                                                 space="PSUM"))
        s_ps = ctx.enter_context(tc.tile_pool(name="s_ps", bufs=3, space="PSUM"))
        o_ps = ctx.enter_context(tc.tile_pool(name="o_ps", bufs=2, space="PSUM"))

        def emit_qk_transpose(i, yqk):
            """One XBAR transpose: y_qk [s, q|k 1024] -> yt [d, 2KC, s]."""
            yt = yt_pool.tile([P, 2 * D], bf16, tag="yt")
            nc.sync.dma_start_transpose(
                yt[:].rearrange("p (c s) -> p c s", c=2 * KC), yqk[:])
            return yt

        def emit_qk_convert(i, yt):
            """Lagged bf16 -> fp8 convert into the persistent q8k8."""
            dst8 = q8k8[:, :, i * P:(i + 1) * P]
            src = yt[:].rearrange("p (c s) -> p c s", c=2 * KC)
            if i % 2 == 0:
                nc.vector.tensor_copy(dst8, src)
            else:
                nc.scalar.copy(dst8, src)

        def proj_tile(x_ap, w, i, kind, y_dst):
            # x_ap: [d_in_part, d_chunk * 128] bf16 (transposed input block)
            pr = proj_ps.tile([P, D], f32, tag="proj")
            for c in range(KC):
                nc.tensor.matmul(pr[:], x_ap[:, c * P:(c + 1) * P],
                                 w[:, c, :],
                                 start=(c == 0), stop=(c == KC - 1))
            # the whole eviction chain is high-priority: it recycles the
            # proj PSUM ring, which directly gates the PE
            with tc.high_priority():
                if kind == "v":
                    if i % 2 == 0:
                        nc.scalar.copy(v_sb[:, i, :], pr[:])
                    else:
                        nc.vector.tensor_copy(v_sb[:, i, :], pr[:])
                    return
                bn6 = small.tile([P, 6], f32, tag="bn6")
                nc.vector.bn_stats(bn6[:], pr[:])
                agg = small.tile([P, 2], f32, tag="agg")
                nc.vector.bn_aggr(agg[:], bn6[:])
                # rstd' = exp(-0.5*ln(var+eps) + ln(g*scale)) -- ln/exp only
                lnv = small.tile([P, 1], f32, tag="lnv")
                nc.scalar.activation(lnv[:], agg[:, 1:2], Act.Ln,
                                     bias=eps_sb[:])
                rstd = small.tile([P, 1], f32, tag="rstd")
                lnbias = lnb[:, 0:1] if kind == "q" else lnb[:, 1:2]
                nc.scalar.activation(rstd[:], lnv[:], Act.Exp, scale=-0.5,
                                     bias=lnbias)
                if i % 2 == 0:
                    c1 = small.tile([P, 1], f32, tag="c1")
                    nc.vector.tensor_scalar(c1[:], agg[:, 0:1], rstd[:], -1.0,
                                            op0=Alu.mult, op1=Alu.mult)
                    nc.scalar.activation(y_dst, pr[:], Act.Identity,
                                         bias=c1[:], scale=rstd[:])
                else:
                    nc.vector.tensor_scalar(y_dst, pr[:], agg[:, 0:1],
                                            rstd[:],
                                            op0=Alu.subtract, op1=Alu.mult)

        def s_block(t):
            """QK matmuls, exp, diagonal-triangle zeroing for query tile t.

            No additive mask on the diagonal block: exp runs unmasked (values
            stay finite: |S| <= sqrt(dk)*g^2), then the strictly-upper
            triangle of P's diagonal 128x128 block is zeroed on gpsimd and
            its row-sum contribution computed separately, keeping the
            S-chunk PSUM critical chain at QK->exp only.
            """
            L = P * (t + 1)
            n_chunks = (L + 511) // 512
            p_sb = p_pool.tile([P, S], bf16, tag="p")
            pt_sb = pt_pool.tile([P, S], bf16, tag="pt")
            pt_v = pt_sb[:].rearrange("p (j s) -> p j s", j=NT)
            sums = small.tile([P, KC + 1], f32, tag="sums")
            for c in range(n_chunks):
                w_cols = min(512, L - c * 512)
                sc = s_ps.tile([P, 512], f32, tag="s")
                for u in range(2):
                    nc.tensor.matmul(sc[:, :w_cols],
                                     q8k8[:, 2 * u:2 * u + 2,
                                          t * P:(t + 1) * P],
                                     q8k8[:, KC + 2 * u:KC + 2 * u + 2,
                                          c * 512:c * 512 + w_cols],
                                     start=(u == 0), stop=(u == 1),
                                     perf_mode=DR)
                if has_km:
                    nc.vector.tensor_tensor(sc[:, :w_cols], sc[:, :w_cols],
                                            km[:, c * 512:c * 512 + w_cols],
                                            op=Alu.add)
                diag = c * 512 <= t * P < c * 512 + w_cols
                if diag:
                    off = t * P - c * 512
                    if off > 0:
                        nc.scalar.activation(p_sb[:, c * 512:c * 512 + off],
                                             sc[:, :off], Act.Exp,
                                             bias=expb[:],
                                             accum_out=sums[:, c:c + 1])
                    else:
                        nc.vector.memset(sums[:, c:c + 1], 0.0)
                    nc.scalar.activation(p_sb[:, t * P:(t + 1) * P],
                                         sc[:, off:off + P], Act.Exp,
                                         bias=expb[:])
                    # zero strictly-upper triangle, then add its row sums
                    # (SBUF-only ops -- the otherwise-idle Pool engine)
                    nc.gpsimd.tensor_tensor(p_sb[:, t * P:(t + 1) * P],
                                            p_sb[:, t * P:(t + 1) * P],
                                            ltri01[:], op=Alu.mult)
                    nc.vector.tensor_reduce(sums[:, n_chunks:n_chunks + 1],
                                            p_sb[:, t * P:(t + 1) * P],
                                            axis=mybir.AxisListType.X,
                                            op=Alu.add)
                else:
                    nc.scalar.activation(p_sb[:, c * 512:c * 512 + w_cols],
                                         sc[:, :w_cols], Act.Exp,
                                         bias=expb[:],
                                         accum_out=sums[:, c:c + 1])
                # P^T XBAR transpose for this chunk's 128-blocks (DMA)
                nc.sync.dma_start_transpose(
                    pt_v[:, c * 4:c * 4 + w_cols // P, :],
                    p_sb[:, c * 512:c * 512 + w_cols])
            return p_sb, sums, n_chunks, pt_v

        def pv_block(t, state):
            """PV matmuls + output chain for tile t (two tiles behind S)."""
            p_sb, sums, n_chunks, pt_v = state
            rr = small.tile([P, 1], f32, tag="rr")
            ssum = small.tile([P, 1], f32, tag="ssum")
            nc.vector.tensor_reduce(ssum[:], sums[:, :n_chunks + 1],
                                    axis=mybir.AxisListType.X, op=Alu.add)
            nc.vector.reciprocal(rr[:], ssum[:])

            ops = o_ps.tile([P, D], f32, tag="o")
            for jj in range(t + 1):
                nc.tensor.matmul(ops[:], pt_v[:, jj, :], v_sb[:, jj, :],
                                 start=(jj == 0), stop=(jj == t))

            # out = LN(o / rowsum + xq) [* go + bo]
            xres = xq_sb[:, t * D:(t + 1) * D]
            z = z_pool.tile([P, D], f32, tag="z")
            nc.vector.scalar_tensor_tensor(z[:], ops[:], rr[:], xres,
                                           op0=Alu.mult, op1=Alu.add)
            bn6 = small.tile([P, 6], f32, tag="bn6z")
            nc.vector.bn_stats(bn6[:], z[:])
            agg = small.tile([P, 2], f32, tag="aggz")
            nc.vector.bn_aggr(agg[:], bn6[:])
            lnv = small.tile([P, 1], f32, tag="lnvz")
            nc.scalar.activation(lnv[:], agg[:, 1:2], Act.Ln, bias=eps_sb[:])
            rstd = small.tile([P, 1], f32, tag="rstdz")
            nc.scalar.activation(rstd[:], lnv[:], Act.Exp, scale=-0.5,
                                 bias=lnb[:, 2:3])
            w1 = z_pool.tile([P, D], f32, tag="w1")
            if t % 2 == 0:
                c1 = small.tile([P, 1], f32, tag="c1z")
                nc.vector.tensor_scalar(c1[:], agg[:, 0:1], rstd[:], -1.0,
                                        op0=Alu.mult, op1=Alu.mult)
                nc.scalar.activation(w1[:], z[:], Act.Identity,
                                     bias=c1[:], scale=rstd[:])
            else:
                nc.vector.tensor_scalar(w1[:], z[:], agg[:, 0:1], rstd[:],
                                        op0=Alu.subtract, op1=Alu.mult)
            if has_gobo:
                o_sb = z_pool.tile([P, D], f32, tag="osb")
                nc.vector.tensor_tensor(o_sb[:], w1[:], go_sb, op=Alu.mult)
                nc.vector.tensor_tensor(o_sb[:], o_sb[:], bo_sb, op=Alu.add)
                nc.gpsimd.dma_start(out_d[t * P:(t + 1) * P, :], o_sb[:])
            else:
                nc.gpsimd.dma_start(out_d[t * P:(t + 1) * P, :], w1[:])

        loop_cm = tc.For_i(0, loop_n, 1) if loop_n else None
        if loop_cm is not None:
            loop_cm.__enter__()
        # whole-tensor input loads, two DMAs each (halves arrive sooner;
        # xq last: its reuse semaphore releases only at the previous
        # iteration's final residual read)
        H = NT * D // 2
        for nm in ("xqT", "xkT", "xvT"):
            for h in range(2):
                nc.sync.dma_start(xin[nm][:, h * H:(h + 1) * H],
                                  xin_dram[nm][:, h * H:(h + 1) * H])
        for h in range(2):
            nc.sync.dma_start(xq_sb[:, h * H:(h + 1) * H],
                              xq_d[:, h * H:(h + 1) * H])
        # ---- stage 1 (transposes lag their projections by ~2 tiles so
        #      their data-ready waits never head-block the SP DMA queue;
        #      fp8 converts lag the transposes by another ~2 tiles) ----
        pend_tp = []
        pend_cv = []

        def drain_tp():
            i2, y2 = pend_tp.pop(0)
            pend_cv.append((i2, emit_qk_transpose(i2, y2)))

        def drain_cv():
            i2, yt2 = pend_cv.pop(0)
            emit_qk_convert(i2, yt2)

        for i in range(NT):
            yqk = y_pool.tile([P, 2 * D], bf16, tag="yqk")
            for nm, wkey, kind, y_dst in (
                    ("xqT", "wq", "q", yqk[:, 0:D]),
                    ("xkT", "wk", "k", yqk[:, D:2 * D]),
                    ("xvT", "wv", "v", None)):
                proj_tile(xin[nm][:, i * D:(i + 1) * D], w_r[wkey], i,
                          kind, y_dst)
            pend_tp.append((i, yqk))
            while len(pend_tp) > 2:
                drain_tp()
            while len(pend_cv) > 2:
                drain_cv()
        while pend_tp:
            drain_tp()
        while pend_cv:
            drain_cv()
        # ---- stage 2 (PV lags S by two query tiles so the per-tile
        #      exp -> XBAR-transpose chain is fully hidden) ----
        prevs = []
        for t in range(NT):
            prevs.append((t, s_block(t)))
            if len(prevs) > 2:
                t2, st = prevs.pop(0)
                pv_block(t2, st)
        for t2, st in prevs:
            pv_block(t2, st)
        if loop_cm is not None:
            loop_cm.__exit__(None, None, None)

    nc.compile()
    return nc


def _get_nc(has_km=False, has_gobo=False):
    key = ("nc", bool(has_km), bool(has_gobo))
    if key not in _CACHE:
        _CACHE[key] = _build(has_km, has_gobo=has_gobo)
    return _CACHE[key]


def _pack_xT(x):
    """[S, D] -> [P, NT*D] bf16: col (i*D + c*P + s') = x[i*P+s', c*P+p]."""
    bf = _bf16np()
    return np.ascontiguousarray(
        np.asarray(x, np.float32).reshape(NT, P, KC, P)
        .transpose(3, 0, 2, 1)).reshape(P, NT * D).astype(bf)


def _pack_xq(x):
    """[S, D] -> [P, NT*D] fp32: col (i*D + d) = x[i*P+p, d]."""
    return np.ascontiguousarray(
        np.asarray(x, np.float32).reshape(NT, P, D)
        .transpose(1, 0, 2)).reshape(P, NT * D)


def _bench_inputs(rng):
    """Input map (one core) with the same shapes/dtypes kernel() feeds."""
    f = np.float32
    bf = _bf16np()
    xq = rng.standard_normal((S, D), dtype=f)
    mkT = lambda: _pack_xT(rng.standard_normal((S, D), dtype=f))
    lnb = np.zeros((P, 3), f)
    lnb[:, 0] = -0.5 * math.log(D)
    return {
        "xq": _pack_xq(xq), "xqT": _pack_xT(xq),
        "xkT": mkT(), "xvT": mkT(),
        "wq": rng.standard_normal((D, D), dtype=f).astype(bf),
        "wk": rng.standard_normal((D, D), dtype=f).astype(bf),
        "wv": rng.standard_normal((D, D), dtype=f).astype(bf),
        "lnb": lnb, "gobo": np.ones((P, 2 * D), f),
        "km": np.zeros((P, S), f),
    }


def _fallback(vals, keys, ques, causal_mask, key_mask, Wv, Wk, Wq,
              ln_k_g, ln_k_b, ln_q_g, ln_q_b, ln_o_g, ln_o_b):
    # numpy reference path; used when the inputs fall outside the pattern
    # this kernel is specialized for.
    def ln(x, g, b):
        mu = x.mean(-1, keepdims=True)
        var = ((x - mu) ** 2).mean(-1, keepdims=True)
        return (x - mu) / np.sqrt(var + EPS) * g + b

    x64 = np.float64
    vals, keys, ques = (np.asarray(a) for a in (vals, keys, ques))
    v = vals.astype(x64) @ np.asarray(Wv, x64)
    k = ln(keys.astype(x64) @ np.asarray(Wk, x64), np.asarray(ln_k_g),
           np.asarray(ln_k_b))
    q = ln(ques.astype(x64) @ np.asarray(Wq, x64), np.asarray(ln_q_g),
           np.asarray(ln_q_b))
    a = np.einsum("bqd,bkd->bqk", q, k) / math.sqrt(D)
    a = np.where(causal_mask[None], -np.inf, a)
    a = np.where(key_mask[:, None, :], -np.inf, a)
    a = a - a.max(-1, keepdims=True)
    p = np.exp(a)
    p /= p.sum(-1, keepdims=True)
    o = np.einsum("bqk,bkd->bqd", p, v)
    return np.asarray(ln(o + ques.astype(x64), np.asarray(ln_o_g),
                         np.asarray(ln_o_b)), np.float32)


def _get_runner(has_km, has_gobo):
    """Build (once) a cached sharded-jit executor for the compiled module."""
    key = ("runner", bool(has_km), bool(has_gobo))
    if key in _CACHE:
        return _CACHE[key]

    import jax
    import numpy as _np
    from jax.sharding import Mesh, PartitionSpec
    from jax.experimental.shard_map import shard_map
    from concourse import mybir
    from concourse.bass2jax import (_bass_exec_p, install_neuronx_cc_hook,
                                    partition_id_tensor)

    install_neuronx_cc_hook()
    nc = _get_nc(has_km, has_gobo)

    pname = nc.partition_id_tensor.name if nc.partition_id_tensor else None
    in_names, out_names, out_avals, zero_outs = [], [], [], []
    for alloc in nc.m.functions[0].allocations:
        if not isinstance(alloc, mybir.MemoryLocationSet):
            continue
        name = alloc.memorylocations[0].name
        if alloc.kind == "ExternalInput":
            if name != pname:
                in_names.append(name)
        elif alloc.kind == "ExternalOutput":
            shape = tuple(alloc.tensor_shape)
            dtype = mybir.dt.np(alloc.dtype)
            out_names.append(name)
            out_avals.append(jax.core.ShapedArray(shape, dtype))
            zero_outs.append(_np.zeros((B * shape[0], *shape[1:]), dtype))
    n_params = len(in_names)
    all_in = in_names + out_names
    if pname is not None:
        all_in = all_in + [pname]

    def _body(*args):
        operands = list(args)
        if pname is not None:
            operands.append(partition_id_tensor())
        outs = _bass_exec_p.bind(
            *operands,
            out_avals=tuple(out_avals),
            in_names=tuple(all_in),
            out_names=tuple(out_names),
            lowering_input_output_aliases=(),
            sim_require_finite=True,
            sim_require_nnan=True,
            nc=nc,
        )
        return tuple(outs)

    devices = jax.devices()[:B]
    mesh = Mesh(np.asarray(devices), ("core",))
    donate = tuple(range(n_params, n_params + len(out_names)))
    sharded = jax.jit(
        shard_map(_body, mesh=mesh,
                  in_specs=(PartitionSpec("core"),) * (n_params + len(out_names)),
                  out_specs=(PartitionSpec("core"),) * len(out_names),
                  check_rep=False),
        donate_argnums=donate, keep_unused=True)

    def run(concat_by_name):
        args = [concat_by_name[n] for n in in_names] + list(zero_outs)
        out_arrs = sharded(*args)
        return {n: _np.asarray(out_arrs[i]).reshape(B, *out_avals[i].shape)
                for i, n in enumerate(out_names)}

    _CACHE[key] = run
    return run


def _uniform_pos(v):
    v = np.asarray(v, np.float32)
    return v.size > 0 and np.all(v == v.flat[0]) and v.flat[0] > 0


def kernel(vals, keys, ques, causal_mask, key_mask, Wv, Wk, Wq,
           ln_k_g, ln_k_b, ln_q_g, ln_q_b, ln_o_g, ln_o_b):
    causal_mask = np.asarray(causal_mask)
    key_mask = np.asarray(key_mask)
    ln_q_b = np.asarray(ln_q_b, np.float32)
    ln_k_b = np.asarray(ln_k_b, np.float32)
    ln_o_g = np.asarray(ln_o_g, np.float32)
    ln_o_b = np.asarray(ln_o_b, np.float32)
    # The device kernel is specialized for: standard causal triu mask,
    # b_q == 0 (b_k is dropped exactly -- it shifts each logit row by a
    # per-query constant, which softmax ignores), uniform positive q/k
    # gains (folded into the Exp bias producing rstd).
    ok = (np.array_equal(causal_mask, np.triu(np.ones((S, S), bool), k=1))
          and not ln_q_b.any()
          and float(np.abs(ln_k_b).max(initial=0.0)) < 16.0
          and _uniform_pos(ln_q_g) and _uniform_pos(ln_k_g))
    if not ok:
        return _fallback(vals, keys, ques, causal_mask, key_mask, Wv, Wk, Wq,
                         ln_k_g, ln_k_b, ln_q_g, ln_q_b, ln_o_g, ln_o_b)

    has_km = bool(key_mask.any())
    has_gobo = not (_uniform_pos(ln_o_g) and not ln_o_b.any())
    run = _get_runner(has_km, has_gobo)

    f = np.float32
    bf = _bf16np()

    lnb = np.zeros((P, 3), f)
    lnb[:, 0] = math.log(float(np.asarray(ln_q_g).flat[0]) / math.sqrt(D))
    lnb[:, 1] = math.log(float(np.asarray(ln_k_g).flat[0]))
    lnb[:, 2] = 0.0 if has_gobo else math.log(float(ln_o_g.flat[0]))
    gobo = np.broadcast_to(
        np.concatenate([ln_o_g, ln_o_b]), (P, 2 * D)).copy()
    xq = np.ascontiguousarray(
        np.asarray(ques, f).reshape(B, NT, P, D).transpose(0, 2, 1, 3)
    ).reshape(B * P, NT * D)

    def xT(a):
        # [B, S, D] fp32 -> per-batch partition-major packed transpose,
        # bf16 [B*P, NT*D]: col (i*D + c*P + s') = a[b, i*P+s', c*P+p]
        return np.ascontiguousarray(
            np.asarray(a, f).reshape(B, NT, P, KC, P).transpose(0, 4, 1, 3, 2)
        ).reshape(B * P, NT * D).astype(bf)

    def rep(a):
        return np.concatenate([a] * B, axis=0)

    km_rows = np.where(key_mask, NEG, f(0)).astype(f)          # [B, S]
    km_cat = np.repeat(km_rows, P, axis=0)                      # [B*P, S]
    concat = {
        "xq": xq,
        "xqT": xT(ques), "xkT": xT(keys), "xvT": xT(vals),
        "wq": rep(np.ascontiguousarray(Wq, f).astype(bf)),
        "wk": rep(np.ascontiguousarray(Wk, f).astype(bf)),
        "wv": rep(np.ascontiguousarray(Wv, f).astype(bf)),
        "lnb": rep(lnb), "gobo": rep(gobo),
        "km": km_cat,
    }
    out = run(concat)["out"]                                    # [B, S, D]
    return out

